# revision 1
# baseline (speedup 1.0000x reference)
"""Trainium2 Bass kernel for nn_AttnBlock (GroupNorm + single-head attention
block over [b=4, c=512, l=4096] fp32, 8 NeuronCores).

Sharding: core = (batch, query-half). Each core gets one batch item with its
query half permuted to columns 0..2047 (GroupNorm/attention are invariant to
a consistent permutation of l), computes the full block for its 2048 query
positions, and the host reassembles the [4, 512, 4096] output.

On-chip per core:
  - GroupNorm stats streamed from HBM (sum via DVE reduce, sum-sq via ACT
    Square+accum), group combine + broadcast via tiny TensorE matmuls with
    group-indicator matrices.
  - GN is folded into the QKV weights (w' = w * diag(m), bias fixups), the
    1/sqrt(c) attention scale folded into wq'.
  - Q/K [c, l] and V^T [l, c] computed as bf16 matmuls (x streamed again).
  - S^T = K^T Q per (i-block 512, j-tile 128); softmax without max-subtract
    (|S| <= ~6 for this model, exp stays in fp32 range), exp on ACT -> bf16;
    row sums s[i] via ones-vector matmuls; O_u = vT.T @ expS^T accumulated
    over j; proj with rank-1 bias inject (bp''' (x) s); normalize by 1/s
    after proj; residual re-DMA'd + added; DMA out.
"""
import os
import sys
from contextlib import ExitStack

import numpy as np

sys.path.insert(0, "/opt/trn_rl_repo")

import concourse.bass as bass
import concourse.tile as tile
from concourse import bacc, mybir

F32 = mybir.dt.float32
BF16 = mybir.dt.bfloat16
F8 = mybir.dt.float8e4

B, C, L = 4, 512, 4096
NQ = L // 2          # queries per core
P = 128
CO = C // P          # 4 channel blocks
NJT = L // P         # 32 j-tiles
NIB = NQ // 512      # 4 i-blocks
NLC = L // 512       # 8 l-chunks
NG = 32              # groups
GSZ = C // NG        # 16 channels per group
GPP = P // GSZ       # 8 groups per 128 partitions
EPS = 1e-6
SCALE = float(C) ** -0.5


def build_program():
    nc = bacc.Bacc("TRN2")
    x_d = nc.declare_dram_parameter("x", [C, L], F32, isOutput=False)
    wq_d = nc.declare_dram_parameter("wqT", [C, C], F32, isOutput=False)
    wk_d = nc.declare_dram_parameter("wkT", [C, C], F32, isOutput=False)
    wv_d = nc.declare_dram_parameter("wvT", [C, C], F32, isOutput=False)
    wp_d = nc.declare_dram_parameter("wpT", [C, C], F32, isOutput=False)
    gns_d = nc.declare_dram_parameter("gn_scale", [C], F32, isOutput=False)
    gnb_d = nc.declare_dram_parameter("gn_bias", [C], F32, isOutput=False)
    bq_d = nc.declare_dram_parameter("bq", [C], F32, isOutput=False)
    bv_d = nc.declare_dram_parameter("bv", [C], F32, isOutput=False)
    bp_d = nc.declare_dram_parameter("bp", [C], F32, isOutput=False)
    gm_d = nc.declare_dram_parameter("gmat", [P, GPP], F32, isOutput=False)
    gt_d = nc.declare_dram_parameter("gtmat", [GPP, P], F32, isOutput=False)
    out_d = nc.declare_dram_parameter("out", [C, NQ], F32, isOutput=True)

    with tile.TileContext(nc) as tc:
        attn_block(tc, x_d, wq_d, wk_d, wv_d, wp_d, gns_d, gnb_d,
                   bq_d, bv_d, bp_d, gm_d, gt_d, out_d)
    nc.compile()
    return nc


def attn_block(tc, x_d, wq_d, wk_d, wv_d, wp_d, gns_d, gnb_d, bq_d, bv_d,
               bp_d, gm_d, gt_d, out_d):
    nc = tc.nc
    x_v = x_d.ap().rearrange("(o p) l -> p o l", p=P)
    out_v = out_d.ap().rearrange("(o p) i -> p o i", p=P)

    with ExitStack() as ctx:
        # ---- persistent pools (whole kernel) ----
        big = ctx.enter_context(tc.tile_pool(name="big", bufs=1))
        wbp = ctx.enter_context(tc.tile_pool(name="wbp", bufs=1))
        small = ctx.enter_context(tc.tile_pool(name="small", bufs=1))
        ps = ctx.enter_context(tc.tile_pool(name="ps", bufs=3, space="PSUM"))

        q_sb = big.tile([P, 2, 2, NQ], F8, tag="qsb")
        k_sb = big.tile([P, 2, 2, L], F8, tag="ksb")
        vt_sb = big.tile([P, NJT // 2, 2, C], F8, tag="vtsb")
        wq_b = wbp.tile([P, 2, 2, C], F8, tag="wqb")
        wk_b = wbp.tile([P, 2, 2, C], F8, tag="wkb")
        wv_b = wbp.tile([P, 2, 2, C], F8, tag="wvb")
        wp_b = wbp.tile([P, 2, 2, C], F8, tag="wpb")

        gns = small.tile([P, CO], F32, tag="gns")
        gnb = small.tile([P, CO], F32, tag="gnb")
        bq_s = small.tile([P, CO], F32, tag="bqs")
        bv_s = small.tile([P, CO], F32, tag="bvs")
        for v_d, v_t in ((gns_d, gns), (gnb_d, gnb), (bq_d, bq_s), (bv_d, bv_s)):
            nc.sync.dma_start(out=v_t[:], in_=v_d.ap().rearrange(
                "(o p) -> p o", p=P))
        bp_s = small.tile([1, C], F32, tag="bps")
        nc.sync.dma_start(out=bp_s[:], in_=bp_d.ap().rearrange("(u c) -> u c", u=1))

        bq2 = small.tile([P, CO], F32, tag="bq2")
        bp3_b = small.tile([1, C], BF16, tag="bp3b")
        ones_p = small.tile([P, 2, 16], F8, tag="onesp")
        nc.vector.memset(ones_p, 1.0)
        nshift = small.tile([P, 1], F32, tag="nshift")
        nc.vector.memset(nshift, -3.0)
        ones_1 = small.tile([1, P], F32, tag="ones1")
        nc.vector.memset(ones_1, 1.0)

        # ========== prologue: stats + bf16 cast + folded weights + QKV ==========
        with ExitStack() as pctx:
            xf_pool = pctx.enter_context(tc.tile_pool(name="xfp", bufs=3))
            wf_pool = pctx.enter_context(tc.tile_pool(name="wfp", bufs=1))
            wfs_pool = pctx.enter_context(tc.tile_pool(name="wfsp", bufs=1))
            pro = pctx.enter_context(tc.tile_pool(name="pro", bufs=1))
            xb_pool = pctx.enter_context(tc.tile_pool(name="xbp", bufs=1))

            x_f8 = xb_pool.tile([P, 2, 2, L], F8, tag="xf8")

            # ---- streamed GroupNorm stats + x -> bf16 cast ----
            # One DMA per channel block o: [128, 4096] with 16 KB contiguous
            # per partition line (max DMA efficiency), striped across both
            # HWDGE rings (SP + ACT).
            wk_f = wf_pool.tile([P, CO, C], F32, tag="wkf")
            wk_v = wk_d.ap().rearrange("(o p) c -> p o c", p=P)
            nc.scalar.dma_start(out=wk_f[:], in_=wk_v[:])

            bnst = pro.tile([P, CO, NLC, 6], F32, tag="bnst")
            HC = L // 2
            for ci, (o, hh) in enumerate((o, hh) for o in range(CO)
                                         for hh in range(2)):
                l0 = hh * HC
                xf = xf_pool.tile([P, HC], F32, tag="xf")
                eng = nc.sync if ci % 2 == 0 else nc.scalar
                eng.dma_start(out=xf[:], in_=x_v[:, o, l0 : l0 + HC])
                for h in range(HC // 512):
                    nc.vector.bn_stats(out=bnst[:, o, hh * (HC // 512) + h, :],
                                       in_=xf[:, h * 512 : (h + 1) * 512])
                nc.scalar.activation(out=x_f8[:, o // 2, o % 2, l0 : l0 + HC],
                                     in_=xf[:],
                                     func=mybir.ActivationFunctionType.Copy)
            wq_f = wfs_pool.tile([P, CO, C], F32, tag="wfs2")
            wq_v = wq_d.ap().rearrange("(o p) c -> p o c", p=P)
            nc.sync.dma_start(out=wq_f[:], in_=wq_v[:])
            wv_f = wfs_pool.tile([P, CO, C], F32, tag="wfs")
            wv_v = wv_d.ap().rearrange("(o p) c -> p o c", p=P)
            nc.scalar.dma_start(out=wv_f[:], in_=wv_v[:])
            mv = pro.tile([P, CO, 2], F32, tag="mv")
            for o in range(CO):
                nc.vector.bn_aggr(out=mv[:, o, :], in_=bnst[:, o, :, :])
            # st8[:, 0:CO] = per-channel mean, st8[:, CO:] = per-channel E[x^2]
            st8 = small.tile([P, 2 * CO], F32, tag="st8")
            nc.vector.tensor_copy(st8[:, 0:CO], mv[:, :, 0])
            nc.vector.tensor_mul(st8[:, CO : 2 * CO], mv[:, :, 0], mv[:, :, 0])
            nc.vector.tensor_add(st8[:, CO : 2 * CO], st8[:, CO : 2 * CO],
                                 mv[:, :, 1])

            # ---- group combine: G[p, j] = (p // 16 == j) / 16 ----
            g_mat = pro.tile([P, GPP], F32, tag="gmat")
            nc.sync.dma_start(out=g_mat[:], in_=gm_d.ap())
            gstat_ps = ps.tile([GPP, 2 * CO], F32, tag="mm")
            nc.tensor.matmul(gstat_ps, lhsT=g_mat, rhs=st8, start=True, stop=True)
            mr8 = pro.tile([GPP, 2 * CO], F32, tag="mr8")
            nc.vector.tensor_copy(mr8[:, 0:CO], gstat_ps[:, 0:CO])
            var8 = pro.tile([GPP, CO], F32, tag="var8")
            nc.vector.tensor_mul(var8, mr8[:, 0:CO], mr8[:, 0:CO])
            nc.vector.tensor_sub(var8, gstat_ps[:, CO : 2 * CO], var8)
            eps_t = pro.tile([GPP, 1], F32, tag="eps")
            nc.vector.memset(eps_t, EPS)
            sq8 = pro.tile([GPP, CO], F32, tag="sq8")
            nc.scalar.activation(out=sq8, in_=var8,
                                 func=mybir.ActivationFunctionType.Sqrt,
                                 bias=eps_t)
            rscr = pro.tile([GPP, CO], F32, tag="rscr")
            nc.vector.reciprocal_approx_accurate(mr8[:, CO : 2 * CO], sq8, rscr)
            gt_mat = pro.tile([GPP, P], F32, tag="gtmat")
            nc.sync.dma_start(out=gt_mat[:], in_=gt_d.ap())
            bc_ps = ps.tile([P, 2 * CO], F32, tag="mm")
            nc.tensor.matmul(bc_ps, lhsT=gt_mat, rhs=mr8, start=True, stop=True)
            m44 = small.tile([P, CO], F32, tag="m44")
            nc.vector.tensor_mul(m44, bc_ps[:, CO : 2 * CO], gns)
            a44 = pro.tile([P, CO], F32, tag="a44")
            nc.vector.tensor_mul(a44, bc_ps[:, 0:CO], m44)
            nc.vector.tensor_sub(a44, gnb, a44)
            mq44 = pro.tile([P, CO], F32, tag="mq44")
            nc.vector.tensor_scalar_mul(mq44, m44, SCALE)

            # ---- folded weights (f32 -> bf16, per-in-channel scale) ----
            # wk first (k matmuls start first); split DVE/ACT for latency
            for o in range(CO):
                if o % 2 == 0:
                    nc.vector.tensor_scalar_mul(wk_b[:, o // 2, o % 2, :],
                                                wk_f[:, o, :],
                                                m44[:, o : o + 1])
                else:
                    nc.scalar.activation(out=wk_b[:, o // 2, o % 2, :],
                                         in_=wk_f[:, o, :],
                                         func=mybir.ActivationFunctionType.Copy,
                                         scale=m44[:, o : o + 1])
            for o in range(CO):
                if o % 2 == 0:
                    nc.vector.tensor_scalar_mul(wv_b[:, o // 2, o % 2, :],
                                                wv_f[:, o, :],
                                                m44[:, o : o + 1])
                else:
                    nc.scalar.activation(out=wv_b[:, o // 2, o % 2, :],
                                         in_=wv_f[:, o, :],
                                         func=mybir.ActivationFunctionType.Copy,
                                         scale=m44[:, o : o + 1])
            for o in range(CO):
                if o % 2 == 0:
                    nc.vector.tensor_scalar_mul(wq_b[:, o // 2, o % 2, :],
                                                wq_f[:, o, :],
                                                m44[:, o : o + 1])
                else:
                    nc.scalar.activation(out=wq_b[:, o // 2, o % 2, :],
                                         in_=wq_f[:, o, :],
                                         func=mybir.ActivationFunctionType.Copy,
                                         scale=m44[:, o : o + 1])
            wp_f = wf_pool.tile([P, CO, C], F32, tag="wkf")
            wp_v = wp_d.ap().rearrange("(o p) c -> p o c", p=P)
            nc.sync.dma_start(out=wp_f[:], in_=wp_v[:])
            for o in range(CO):
                nc.vector.tensor_copy(wp_b[:, o // 2, o % 2, :], wp_f[:, o, :])

            # ---- bias fixups ----
            # bq'' = (bq + wq @ a) * scale ; bv'' = bv + wv @ a
            bv2 = pro.tile([P, CO], F32, tag="bv2")
            for dst, w_t, b_t, sc in ((bq2, wq_f, bq_s, 1.0),
                                      (bv2, wv_f, bv_s, 1.0)):
                for oc in range(CO):
                    mv_ps = ps.tile([P, 1], F32, tag="mm")
                    for cc in range(CO):
                        nc.tensor.matmul(mv_ps,
                                         lhsT=w_t[:, cc, oc * P : (oc + 1) * P],
                                         rhs=a44[:, cc : cc + 1],
                                         start=(cc == 0), stop=(cc == CO - 1))
                    nc.vector.tensor_add(dst[:, oc : oc + 1], mv_ps,
                                         b_t[:, oc : oc + 1])
                if sc != 1.0:
                    nc.vector.tensor_scalar_mul(dst, dst, sc)
            bv2_b = pro.tile([P, CO], F8, tag="bv2b")
            nc.vector.tensor_copy(bv2_b, bv2)
            # bp''' = bp + wp'' @ bv''
            bp3_ps = ps.tile([1, C], F32, tag="mm")
            for cc in range(CO):
                nc.tensor.matmul(bp3_ps, lhsT=bv2_b[:, cc : cc + 1],
                                 rhs=wp_b[:, cc // 2, cc % 2, :],
                                 start=(cc == 0), stop=(cc == CO - 1))
            bp3_f = pro.tile([1, C], F32, tag="bp3f")
            nc.vector.tensor_add(bp3_f, bp3_ps, bp_s)
            nc.vector.tensor_copy(bp3_b, bp3_f)

            # ---- Q / K / V^T from resident x_bf ----
            DR = mybir.MatmulPerfMode.DoubleRow
            for lc in range(NLC):
                l0 = lc * 512
                for oc in range(CO):
                    kp = ps.tile([P, 512], F32, tag="mm")
                    for pr in range(2):
                        nc.tensor.matmul(
                            kp, lhsT=wk_b[:, pr, :, oc * P : (oc + 1) * P],
                            rhs=x_f8[:, pr, :, l0 : l0 + 512],
                            start=(pr == 0), stop=(pr == 1), perf_mode=DR)
                    nc.scalar.activation(
                        out=k_sb[:, oc // 2, oc % 2, l0 : l0 + 512], in_=kp,
                        func=mybir.ActivationFunctionType.Copy)
                for jt in range(4):
                    j0 = l0 + jt * P
                    jtg = lc * 4 + jt
                    vp = ps.tile([P, C], F32, tag="mm")
                    for pr in range(2):
                        nc.tensor.matmul(
                            vp, lhsT=x_f8[:, pr, :, j0 : j0 + P],
                            rhs=wv_b[:, pr, :, :],
                            start=(pr == 0), stop=(pr == 1), perf_mode=DR)
                    nc.vector.tensor_copy(vt_sb[:, jtg // 2, jtg % 2, :], vp)
                if lc < NIB:
                    for oc in range(CO):
                        qp = ps.tile([P, 512], F32, tag="mm")
                        for pr in range(2):
                            nc.tensor.matmul(
                                qp, lhsT=wq_b[:, pr, :, oc * P : (oc + 1) * P],
                                rhs=x_f8[:, pr, :, l0 : l0 + 512],
                                start=(pr == 0), stop=(pr == 1), perf_mode=DR)
                        nc.vector.tensor_scalar_add(
                            q_sb[:, oc // 2, oc % 2, l0 : l0 + 512], qp,
                            bq2[:, oc : oc + 1])

        # ================= attention + proj per i-block =================
        with ExitStack() as actx:
            p_pool = actx.enter_context(tc.tile_pool(name="ppool", bufs=4))
            osb_pool = actx.enter_context(tc.tile_pool(name="osb", bufs=2))
            out_pool = actx.enter_context(tc.tile_pool(name="outp", bufs=4))
            res_pool = actx.enter_context(tc.tile_pool(name="resp", bufs=4))
            tiny = actx.enter_context(tc.tile_pool(name="tiny", bufs=2))
            ps_o = actx.enter_context(
                tc.tile_pool(name="pso", bufs=4, space="PSUM"))
            ps_s = actx.enter_context(
                tc.tile_pool(name="pss", bufs=1, space="PSUM"))

            for ib in range(NIB):
                i0 = ib * 512
                s_ps = ps_s.tile([16, 512], F32, tag="srow")
                o_ps = [ps_o.tile([P, 512], F32, tag="oacc", name=f"oacc{cc}")
                        for cc in range(CO)]
                NT = NJT // 2
                for t in range(NT):
                    p_f8 = p_pool.tile([P, 2, 512], F8, tag="pbf")
                    for ko in range(2):
                        jt = 2 * t + ko
                        st_ps = ps.tile([P, 512], F32, tag="mm")
                        for pr in range(2):
                            nc.tensor.matmul(
                                st_ps,
                                lhsT=k_sb[:, pr, :, jt * P : (jt + 1) * P],
                                rhs=q_sb[:, pr, :, i0 : i0 + 512],
                                start=(pr == 0), stop=(pr == 1),
                                perf_mode=mybir.MatmulPerfMode.DoubleRow)
                        # exp(S/sqrt(c) - 3): the attention scale rides the
                        # ACT scale input; the -3 shift keeps P inside fp8e4
                        # range and cancels in the 1/s normalization and the
                        # bp''' (x) s inject.
                        nc.scalar.activation(
                            out=p_f8[:, ko, :], in_=st_ps,
                            func=mybir.ActivationFunctionType.Exp,
                            bias=nshift, scale=SCALE)
                    nc.tensor.matmul(s_ps, lhsT=ones_p, rhs=p_f8,
                                     start=(t == 0), stop=(t == NT - 1),
                                     perf_mode=mybir.MatmulPerfMode.DoubleRow)
                    for cc in range(CO):
                        nc.tensor.matmul(
                            o_ps[cc],
                            lhsT=vt_sb[:, t, :, cc * P : (cc + 1) * P],
                            rhs=p_f8,
                            start=(t == 0), stop=(t == NT - 1),
                            perf_mode=mybir.MatmulPerfMode.DoubleRow)
                s_f = tiny.tile([1, 512], F32, tag="sf")
                nc.vector.tensor_copy(s_f, s_ps[0:1, :])
                s_b = tiny.tile([1, 512], BF16, tag="sb")
                nc.vector.tensor_scalar_mul(s_b, s_f, 1.0 / 32.0)
                rinv1 = tiny.tile([1, 512], F32, tag="rinv1")
                nc.vector.reciprocal_approx_fast(rinv1, s_f)
                nc.vector.tensor_scalar_mul(rinv1, rinv1, 32.0)
                rb_ps = ps.tile([P, 512], F32, tag="mm")
                nc.tensor.matmul(rb_ps, lhsT=ones_1, rhs=rinv1,
                                 start=True, stop=True)
                rinv_b = tiny.tile([P, 512], F32, tag="rinvb")
                nc.vector.tensor_copy(rinv_b, rb_ps)
                o_sb = osb_pool.tile([P, 2, 2, 512], F8, tag="osb")
                for cc in range(CO):
                    if cc % 2 == 0:
                        nc.vector.tensor_scalar_mul(o_sb[:, cc // 2, cc % 2, :],
                                                    o_ps[cc], 1.0 / 32.0)
                    else:
                        nc.scalar.activation(
                            out=o_sb[:, cc // 2, cc % 2, :], in_=o_ps[cc],
                            func=mybir.ActivationFunctionType.Copy,
                            scale=1.0 / 32.0)
                for oc in range(CO):
                    res = res_pool.tile([P, 512], F32, tag="res")
                    nc.sync.dma_start(out=res, in_=x_v[:, oc, i0 : i0 + 512])
                    pj_ps = ps_o.tile([P, 512], F32, tag="oacc",
                                      name=f"pj{oc}")
                    for pr in range(2):
                        nc.tensor.matmul(
                            pj_ps,
                            lhsT=wp_b[:, pr, :, oc * P : (oc + 1) * P],
                            rhs=o_sb[:, pr, :, :],
                            start=(pr == 0), stop=False,
                            perf_mode=mybir.MatmulPerfMode.DoubleRow)
                    nc.tensor.matmul(pj_ps,
                                     lhsT=bp3_b[:, oc * P : (oc + 1) * P],
                                     rhs=s_b, start=False, stop=True)
                    out_t = out_pool.tile([P, 512], F32, tag="outt")
                    nc.vector.tensor_mul(out_t, pj_ps, rinv_b)
                    nc.vector.tensor_add(out_t, out_t, res)
                    nc.sync.dma_start(out=out_v[:, oc, i0 : i0 + 512], in_=out_t)


def kernel(**inputs):
    x = np.ascontiguousarray(np.asarray(inputs["x"], np.float32))
    args = {
        "wqT": np.ascontiguousarray(np.asarray(inputs["wq"], np.float32).T),
        "wkT": np.ascontiguousarray(np.asarray(inputs["wk"], np.float32).T),
        "wvT": np.ascontiguousarray(np.asarray(inputs["wv"], np.float32).T),
        "wpT": np.ascontiguousarray(np.asarray(inputs["wp"], np.float32).T),
        "gn_scale": np.asarray(inputs["gn_scale"], np.float32),
        "gn_bias": np.asarray(inputs["gn_bias"], np.float32),
        "bq": np.asarray(inputs["bq"], np.float32),
        "bv": np.asarray(inputs["bv"], np.float32),
        "bp": np.asarray(inputs["bp"], np.float32),
    }
    pidx = np.arange(P)
    gmat = (pidx[:, None] // GSZ == np.arange(GPP)[None, :]).astype(np.float32)
    args["gmat"] = np.ascontiguousarray(gmat / float(GSZ))
    args["gtmat"] = np.ascontiguousarray(gmat.T)
    in_maps = []
    for core in range(8):
        bi, half = core // 2, core % 2
        sl = slice(half * NQ, (half + 1) * NQ)
        other = slice((1 - half) * NQ, (2 - half) * NQ)
        xp = np.ascontiguousarray(
            np.concatenate([x[bi][:, sl], x[bi][:, other]], axis=1))
        in_maps.append({"x": xp, **args})

    from concourse.bass_utils import run_bass_kernel_spmd

    nc = build_program()
    trace = bool(int(os.environ.get("KERNEL_TRACE", "0")))
    res = run_bass_kernel_spmd(nc, in_maps, core_ids=list(range(8)),
                               trace=trace)
    kernel.last_results = res
    out = np.empty((B, C, L), np.float32)
    for core in range(8):
        bi, half = core // 2, core % 2
        out[bi][:, half * NQ : (half + 1) * NQ] = res.results[core]["out"]
    return out



# revision 8
# speedup vs baseline: 1.2420x; 1.2420x over previous
"""Trainium2 Bass kernel for nn_AttnBlock (GroupNorm + single-head attention
block over [b=4, c=512, l=4096] fp32, 8 NeuronCores).

Sharding: core = (batch, query-half). Each core gets one batch item with its
query half permuted to columns 0..2047 (GroupNorm/attention are invariant to
a consistent permutation of l), computes the full block for its 2048 query
positions, and the host reassembles the [4, 512, 4096] output.

v2 design (vs baseline):
  - Weights pre-scaled by WS=16 and cast to fp8e4m3 on the HOST: kills 3 MB
    of prologue DMA + all on-chip weight-fold matmuls/casts. Power-of-two
    compensations fold into existing scalar constants.
  - GroupNorm applied to x during the fp8 cast (scale+bias fused into
    ACT Identity / DVE tensor_scalar), per channel-block pipelined stats:
    no weight folding, no bias-fixup matmuls, QKV starts ~20 us earlier.
  - x query-half kept resident in SBUF (f32) for the residual: no re-read.
  - s row-sums via M=128 all-ones stationary => s broadcast to all
    partitions for free; 1/s on DVE directly; no PE broadcast matmul, no
    rank-1 bias-inject matmuls (bp3 added post-normalize on ACT).
  - PE warmup dummies paced by stats chunks to pre-trigger the HAM clock
    un-throttle before the QKV burst.
"""
import os
import sys
from contextlib import ExitStack

import numpy as np

sys.path.insert(0, "/opt/trn_rl_repo")

import concourse.bass as bass
import concourse.tile as tile
from concourse import bacc, mybir

F32 = mybir.dt.float32
F8 = mybir.dt.float8e4

B, C, L = 4, 512, 4096
NQ = L // 2          # queries per core
P = 128
CO = C // P          # 4 channel blocks
NJT = L // P         # 32 j-tiles
NIB = NQ // 512      # 4 i-blocks
NLC = L // 512       # 8 l-chunks
NG = 32              # groups
GSZ = C // NG        # 16 channels per group
GPP = P // GSZ       # 8 groups per 128 partitions
EPS = 1e-6
SCALE = float(C) ** -0.5
WS = 16.0            # host-side weight scale (power of two)
ESCALE = SCALE / (WS * WS)   # exp() input scale
OSH = 1.0 / 32.0     # O_u -> fp8 shrink
RS = 32.0 / (WS * WS)        # rinv compensation: (1/s) * 32 / WS^2
HC = L // 2          # half-columns per x chunk

AF = mybir.ActivationFunctionType
ALU = mybir.AluOpType
DR = mybir.MatmulPerfMode.DoubleRow


def build_program():
    nc = bacc.Bacc("TRN2")
    x_d = nc.declare_dram_parameter("x", [C, L], F32, isOutput=False)
    wq_d = nc.declare_dram_parameter("wq8", [C, C], F8, isOutput=False)
    wk_d = nc.declare_dram_parameter("wk8", [C, C], F8, isOutput=False)
    wv_d = nc.declare_dram_parameter("wv8", [C, C], F8, isOutput=False)
    wp_d = nc.declare_dram_parameter("wp8", [C, C], F8, isOutput=False)
    gns_d = nc.declare_dram_parameter("gn_scale", [C], F32, isOutput=False)
    gnb_d = nc.declare_dram_parameter("gn_bias", [C], F32, isOutput=False)
    bqw_d = nc.declare_dram_parameter("bqw", [C], F32, isOutput=False)
    bv8_d = nc.declare_dram_parameter("bv8", [C], F8, isOutput=False)
    bp_d = nc.declare_dram_parameter("bp", [C], F32, isOutput=False)
    gm_d = nc.declare_dram_parameter("gmat", [P, GPP], F32, isOutput=False)
    gt_d = nc.declare_dram_parameter("gtmat", [GPP, P], F32, isOutput=False)
    out_d = nc.declare_dram_parameter("out", [C, NQ], F32, isOutput=True)

    with tile.TileContext(nc) as tc:
        attn_block(tc, x_d, wq_d, wk_d, wv_d, wp_d, gns_d, gnb_d,
                   bqw_d, bv8_d, bp_d, gm_d, gt_d, out_d)
    nc.compile()
    return nc


def attn_block(tc, x_d, wq_d, wk_d, wv_d, wp_d, gns_d, gnb_d, bqw_d, bv8_d,
               bp_d, gm_d, gt_d, out_d):
    nc = tc.nc
    x_v = x_d.ap().rearrange("(o p) l -> p o l", p=P)
    out_v = out_d.ap().rearrange("(o p) i -> p o i", p=P)

    with ExitStack() as ctx:
        # ---- persistent pools (whole kernel) ----
        big = ctx.enter_context(tc.tile_pool(name="big", bufs=1))
        small = ctx.enter_context(tc.tile_pool(name="small", bufs=1))

        xq = big.tile([P, CO, NQ], F32, tag="xq")      # resident query half
        x8 = big.tile([P, 2, 2, L], F8, tag="x8")      # GN-normalized x, fp8
        k8 = big.tile([P, 2, 2, L], F8, tag="k8")
        q8 = big.tile([P, 2, 2, NQ], F8, tag="q8")
        vt8 = big.tile([P, NJT // 2, 2, C], F8, tag="vt8")
        wq8 = big.tile([P, 2, 2, C], F8, tag="wq8")
        wk8 = big.tile([P, 2, 2, C], F8, tag="wk8")
        wv8 = big.tile([P, 2, 2, C], F8, tag="wv8")
        wp8 = big.tile([P, 2, 2, C], F8, tag="wp8")

        gns = small.tile([P, CO], F32, tag="gns")
        gnb = small.tile([P, CO], F32, tag="gnb")
        bqw = small.tile([P, CO], F32, tag="bqw")
        bp_s = small.tile([P, CO], F32, tag="bps")
        bp3 = small.tile([P, CO], F32, tag="bp3")
        bv8 = small.tile([P, 2, 2], F8, tag="bv8")
        m44 = small.tile([P, CO], F32, tag="m44")
        a44 = small.tile([P, CO], F32, tag="a44")
        gm_t = small.tile([P, GPP], F32, tag="gmt")
        gt_t = small.tile([GPP, P], F32, tag="gtt")
        ones_p = small.tile([P, 2, P], F8, tag="onesp")
        nc.vector.memset(ones_p, 1.0)
        nshift = small.tile([P, 1], F32, tag="nshift")
        nc.vector.memset(nshift, -3.0)
        eps_t = small.tile([GPP, 1], F32, tag="eps")
        nc.vector.memset(eps_t, EPS)
        warm8 = small.tile([P, 512], F8, tag="warm8")
        nc.vector.memset(warm8, 1.0)

        # static PSUM layout: 2 + 2 + 4 = 8 banks, shared across phases
        ps_st = ctx.enter_context(
            tc.tile_pool(name="psst", bufs=2, space="PSUM"))
        ps_s = ctx.enter_context(
            tc.tile_pool(name="pss", bufs=2, space="PSUM"))
        ps_o = ctx.enter_context(
            tc.tile_pool(name="pso", bufs=4, space="PSUM"))

        # small/weight DMAs on the gpsimd ring (x owns the sync ring)
        for v_d, v_t in ((gns_d, gns), (gnb_d, gnb), (bqw_d, bqw),
                         (bp_d, bp_s)):
            nc.gpsimd.dma_start(out=v_t[:], in_=v_d.ap().rearrange(
                "(o p) -> p o", p=P))
        nc.gpsimd.dma_start(out=bv8[:], in_=bv8_d.ap().rearrange(
            "(a b p) -> p a b", a=2, b=2, p=P))
        nc.gpsimd.dma_start(out=gm_t[:], in_=gm_d.ap())
        nc.gpsimd.dma_start(out=gt_t[:], in_=gt_d.ap())
        for w_d, w_t in ((wk_d, wk8), (wv_d, wv8), (wq_d, wq8), (wp_d, wp8)):
            nc.gpsimd.dma_start(out=w_t[:], in_=w_d.ap().rearrange(
                "(a b p) c -> p a b c", a=2, b=2, p=P))

        # ========== prologue: streamed stats -> GN-fused fp8 cast ==========
        with ExitStack() as pctx:
            xh_pool = pctx.enter_context(tc.tile_pool(name="xhp", bufs=2))
            pro = pctx.enter_context(tc.tile_pool(name="pro", bufs=1))
            tiny_ps = ps_st
            qkv_ps = ps_o

            bnst = pro.tile([P, CO, 8, 6], F32, tag="bnst")
            mv = pro.tile([P, CO, 2], F32, tag="mv")
            st2 = pro.tile([P, CO, 2], F32, tag="st2")
            sc2 = pro.tile([P, CO, 1], F32, tag="sc2")
            grp = pro.tile([GPP, CO, 6], F32, tag="grp")

            xh_tiles = {}
            ci = 0
            for o in range(CO):
                for hh in range(2):
                    l0 = hh * HC
                    if hh == 0:
                        xf = xq[:, o, :]
                    else:
                        xh = xh_pool.tile([P, HC], F32, tag="xh",
                                          name=f"xh{o}")
                        xh_tiles[o] = xh
                        xf = xh[:]
                    nc.sync.dma_start(out=xf, in_=x_v[:, o, l0:l0 + HC])
                    for h in range(4):
                        nc.vector.bn_stats(
                            out=bnst[:, o, hh * 4 + h, :],
                            in_=xf[:, h * 512:(h + 1) * 512])
                    # HAM warmup: dummy matmul paced by this chunk's stats
                    nc.vector.tensor_copy(warm8[:, ci * 4:ci * 4 + 4],
                                          bnst[:, o, hh * 4 + 3, 0:4])
                    wm_ps = tiny_ps.tile([P, 512], F32, tag="mm")
                    nc.tensor.matmul(wm_ps, lhsT=warm8[:, 0:P],
                                     rhs=warm8[:], start=True, stop=True)
                    ci += 1

                # ---- block o complete: group stats -> m/a -> fp8 cast ----
                nc.vector.bn_aggr(out=mv[:, o, :], in_=bnst[:, o, :, :])
                nc.vector.tensor_copy(st2[:, o, 0:1], mv[:, o, 0:1])
                nc.vector.tensor_mul(sc2[:, o, :], mv[:, o, 0:1],
                                     mv[:, o, 0:1])
                nc.vector.tensor_add(st2[:, o, 1:2], sc2[:, o, :],
                                     mv[:, o, 1:2])
                g_ps = tiny_ps.tile([GPP, 2], F32, tag="mm")
                nc.tensor.matmul(g_ps, lhsT=gm_t, rhs=st2[:, o, :],
                                 start=True, stop=True)
                # var = E[x^2] - mean^2 ; rstd = 1/sqrt(var+eps)
                nc.vector.tensor_copy(grp[:, o, 0:1], g_ps[:, 0:1])
                nc.vector.tensor_mul(grp[:, o, 2:3], grp[:, o, 0:1],
                                     grp[:, o, 0:1])
                nc.vector.tensor_sub(grp[:, o, 2:3], g_ps[:, 1:2],
                                     grp[:, o, 2:3])
                nc.scalar.activation(out=grp[:, o, 3:4], in_=grp[:, o, 2:3],
                                     func=AF.Sqrt, bias=eps_t)
                nc.vector.reciprocal_approx_accurate(
                    grp[:, o, 1:2], grp[:, o, 3:4], grp[:, o, 4:5])
                bc_ps = tiny_ps.tile([P, 2], F32, tag="mm")
                nc.tensor.matmul(bc_ps, lhsT=gt_t, rhs=grp[:, o, 0:2],
                                 start=True, stop=True)
                mcol = m44[:, o:o + 1]
                acol = a44[:, o:o + 1]
                nc.vector.tensor_mul(mcol, bc_ps[:, 1:2], gns[:, o:o + 1])
                nc.vector.tensor_mul(acol, bc_ps[:, 0:1], mcol)
                nc.vector.tensor_sub(acol, gnb[:, o:o + 1], acol)
                # fused GN cast: xhat = m*x + a, split ACT/DVE
                for hh in range(2):
                    src = xq[:, o, :] if hh == 0 else xh_tiles[o][:]
                    dst = x8[:, o // 2, o % 2, hh * HC:(hh + 1) * HC]
                    nc.scalar.activation(out=dst[:, 0:1024],
                                         in_=src[:, 0:1024],
                                         func=AF.Identity,
                                         bias=acol, scale=mcol)
                    nc.vector.tensor_scalar(out=dst[:, 1024:2048],
                                            in0=src[:, 1024:2048],
                                            scalar1=mcol, scalar2=acol,
                                            op0=ALU.mult, op1=ALU.add)

            # ---- bp3 = bp + (wp @ bv) : post-normalize bias ----
            for oc in range(CO):
                b_ps = tiny_ps.tile([P, 1], F32, tag="mm")
                for o in range(CO):
                    nc.tensor.matmul(b_ps,
                                     lhsT=wp8[:, o // 2, o % 2,
                                              oc * P:(oc + 1) * P],
                                     rhs=bv8[:, o // 2, o % 2:o % 2 + 1],
                                     start=(o == 0), stop=(o == CO - 1))
                nc.vector.tensor_scalar(out=bp3[:, oc:oc + 1], in0=b_ps,
                                        scalar1=1.0 / (WS * WS),
                                        scalar2=bp_s[:, oc:oc + 1],
                                        op0=ALU.mult, op1=ALU.add)

            # ---- Q / K / V^T from resident GN-fp8 x ----
            ev = 0
            for lc in range(NLC):
                l0 = lc * 512
                for oc in range(CO):
                    kp = qkv_ps.tile([P, 512], F32, tag="acc")
                    for pr in range(2):
                        nc.tensor.matmul(
                            kp, lhsT=wk8[:, pr, :, oc * P:(oc + 1) * P],
                            rhs=x8[:, pr, :, l0:l0 + 512],
                            start=(pr == 0), stop=(pr == 1), perf_mode=DR)
                    dst = k8[:, oc // 2, oc % 2, l0:l0 + 512]
                    if ev % 2 == 0:
                        nc.scalar.activation(out=dst, in_=kp, func=AF.Copy)
                    else:
                        nc.vector.tensor_copy(dst, kp)
                    ev += 1
                for jt in range(4):
                    j0 = l0 + jt * P
                    jtg = lc * 4 + jt
                    vp = qkv_ps.tile([P, C], F32, tag="acc")
                    for pr in range(2):
                        nc.tensor.matmul(
                            vp, lhsT=x8[:, pr, :, j0:j0 + P],
                            rhs=wv8[:, pr, :, :],
                            start=(pr == 0), stop=(pr == 1), perf_mode=DR)
                    dst = vt8[:, jtg // 2, jtg % 2, :]
                    if ev % 2 == 0:
                        nc.scalar.activation(out=dst, in_=vp, func=AF.Copy)
                    else:
                        nc.vector.tensor_copy(dst, vp)
                    ev += 1
                if lc < NIB:
                    for oc in range(CO):
                        qp = qkv_ps.tile([P, 512], F32, tag="acc")
                        for pr in range(2):
                            nc.tensor.matmul(
                                qp, lhsT=wq8[:, pr, :, oc * P:(oc + 1) * P],
                                rhs=x8[:, pr, :, l0:l0 + 512],
                                start=(pr == 0), stop=(pr == 1), perf_mode=DR)
                        nc.vector.tensor_scalar_add(
                            q8[:, oc // 2, oc % 2, l0:l0 + 512], qp,
                            bqw[:, oc:oc + 1])

        # ================= attention + proj per i-block =================
        with ExitStack() as actx:
            p_pool = actx.enter_context(tc.tile_pool(name="ppool", bufs=4))
            osb_pool = actx.enter_context(tc.tile_pool(name="osb", bufs=2))
            out_pool = actx.enter_context(tc.tile_pool(name="outp", bufs=4))
            rinv_pool = actx.enter_context(tc.tile_pool(name="rinvp", bufs=2))

            NT = NJT // 2
            for ib in range(NIB):
                i0 = ib * 512
                s_ps = ps_s.tile([P, 512], F32, tag="srow")
                o_ps = [ps_o.tile([P, 512], F32, tag="acc", name=f"oacc{cc}")
                        for cc in range(CO)]
                for t in range(NT):
                    p_f8 = p_pool.tile([P, 2, 512], F8, tag="pbf")
                    for ko in range(2):
                        jt = 2 * t + ko
                        st_ps = ps_st.tile([P, 512], F32, tag="mm")
                        for pr in range(2):
                            nc.tensor.matmul(
                                st_ps,
                                lhsT=k8[:, pr, :, jt * P:(jt + 1) * P],
                                rhs=q8[:, pr, :, i0:i0 + 512],
                                start=(pr == 0), stop=(pr == 1),
                                perf_mode=DR)
                        # exp(S/sqrt(c) - 3): shift keeps P in fp8e4 range,
                        # cancels between the s-normalization and bp3 path.
                        nc.scalar.activation(
                            out=p_f8[:, ko, :], in_=st_ps, func=AF.Exp,
                            bias=nshift, scale=ESCALE)
                    nc.tensor.matmul(s_ps, lhsT=ones_p, rhs=p_f8,
                                     start=(t == 0), stop=(t == NT - 1),
                                     perf_mode=DR)
                    for cc in range(CO):
                        nc.tensor.matmul(
                            o_ps[cc],
                            lhsT=vt8[:, t, :, cc * P:(cc + 1) * P],
                            rhs=p_f8,
                            start=(t == 0), stop=(t == NT - 1),
                            perf_mode=DR)
                # ---- epilogue: 1/s, fp8 O, proj, residual ----
                rinv = rinv_pool.tile([P, 512], F32, tag="rinv")
                nc.vector.reciprocal_approx_fast(rinv, s_ps)
                nc.vector.tensor_scalar_mul(rinv, rinv, RS)
                o_sb = osb_pool.tile([P, 2, 2, 512], F8, tag="osb")
                for cc in range(CO):
                    dst = o_sb[:, cc // 2, cc % 2, :]
                    if cc % 2 == 0:
                        nc.scalar.activation(out=dst, in_=o_ps[cc],
                                             func=AF.Copy, scale=OSH)
                    else:
                        nc.vector.tensor_scalar_mul(dst, o_ps[cc], OSH)
                for oc in range(CO):
                    pj_ps = ps_o.tile([P, 512], F32, tag="acc",
                                      name=f"pj{oc}")
                    for pr in range(2):
                        nc.tensor.matmul(
                            pj_ps,
                            lhsT=wp8[:, pr, :, oc * P:(oc + 1) * P],
                            rhs=o_sb[:, pr, :, :],
                            start=(pr == 0), stop=(pr == 1), perf_mode=DR)
                    out_t = out_pool.tile([P, 512], F32, tag="outt")
                    nc.vector.tensor_mul(out_t, pj_ps, rinv)
                    nc.scalar.activation(out=out_t, in_=out_t,
                                         func=AF.Identity,
                                         bias=bp3[:, oc:oc + 1])
                    nc.vector.tensor_add(out_t, out_t,
                                         xq[:, oc, i0:i0 + 512])
                    nc.sync.dma_start(out=out_v[:, oc, i0:i0 + 512],
                                      in_=out_t)


def kernel(**inputs):
    import ml_dtypes

    F8NP = ml_dtypes.float8_e4m3fn
    x = np.ascontiguousarray(np.asarray(inputs["x"], np.float32))
    args = {}
    for nm, w in (("wq8", inputs["wq"]), ("wk8", inputs["wk"]),
                  ("wv8", inputs["wv"]), ("wp8", inputs["wp"])):
        wT = np.asarray(w, np.float32).T * WS
        args[nm] = np.ascontiguousarray(wT.astype(F8NP))
    args["gn_scale"] = np.asarray(inputs["gn_scale"], np.float32)
    args["gn_bias"] = np.asarray(inputs["gn_bias"], np.float32)
    args["bqw"] = np.asarray(inputs["bq"], np.float32) * np.float32(WS)
    args["bv8"] = (np.asarray(inputs["bv"], np.float32)
                   * np.float32(WS)).astype(F8NP)
    args["bp"] = np.asarray(inputs["bp"], np.float32)
    pidx = np.arange(P)
    gmat = (pidx[:, None] // GSZ == np.arange(GPP)[None, :]).astype(np.float32)
    args["gmat"] = np.ascontiguousarray(gmat / float(GSZ))
    args["gtmat"] = np.ascontiguousarray(gmat.T)
    in_maps = []
    for core in range(8):
        bi, half = core // 2, core % 2
        sl = slice(half * NQ, (half + 1) * NQ)
        other = slice((1 - half) * NQ, (2 - half) * NQ)
        xp = np.ascontiguousarray(
            np.concatenate([x[bi][:, sl], x[bi][:, other]], axis=1))
        in_maps.append({"x": xp, **args})

    from concourse.bass_utils import run_bass_kernel_spmd

    nc = build_program()
    trace = bool(int(os.environ.get("KERNEL_TRACE", "0")))
    res = run_bass_kernel_spmd(nc, in_maps, core_ids=list(range(8)),
                               trace=trace)
    kernel.last_results = res
    out = np.empty((B, C, L), np.float32)
    for core in range(8):
        bi, half = core // 2, core % 2
        out[bi][:, half * NQ:(half + 1) * NQ] = res.results[core]["out"]
    return out


# revision 13
# speedup vs baseline: 1.3974x; 1.1251x over previous
"""Trainium2 Bass kernel for nn_AttnBlock (GroupNorm + single-head attention
block over [b=4, c=512, l=4096] fp32, 8 NeuronCores).

Sharding: core = (batch, query-half). Each core gets one batch item with its
query half permuted to columns 0..2047 (GroupNorm/attention are invariant to
a consistent permutation of l), computes the full block for its 2048 query
positions, and the host reassembles the [4, 512, 4096] output.

v2 design (vs baseline):
  - Weights pre-scaled by WS=16 and cast to fp8e4m3 on the HOST: kills 3 MB
    of prologue DMA + all on-chip weight-fold matmuls/casts. Power-of-two
    compensations fold into existing scalar constants.
  - GroupNorm applied to x during the fp8 cast (scale+bias fused into
    ACT Identity / DVE tensor_scalar), per channel-block pipelined stats:
    no weight folding, no bias-fixup matmuls, QKV starts ~20 us earlier.
  - x query-half kept resident in SBUF (f32) for the residual: no re-read.
  - s row-sums via M=128 all-ones stationary => s broadcast to all
    partitions for free; 1/s on DVE directly; no PE broadcast matmul, no
    rank-1 bias-inject matmuls (bp3 added post-normalize on ACT).
  - PE warmup dummies paced by stats chunks to pre-trigger the HAM clock
    un-throttle before the QKV burst.
"""
import os
import sys
from contextlib import ExitStack

import numpy as np

sys.path.insert(0, "/opt/trn_rl_repo")

import concourse.bass as bass
import concourse.tile as tile
from concourse import bacc, mybir

F32 = mybir.dt.float32
F8 = mybir.dt.float8e4

B, C, L = 4, 512, 4096
NQ = L // 2          # queries per core
P = 128
CO = C // P          # 4 channel blocks
NJT = L // P         # 32 j-tiles
NIB = NQ // 512      # 4 i-blocks
NLC = L // 512       # 8 l-chunks
NG = 32              # groups
GSZ = C // NG        # 16 channels per group
GPP = P // GSZ       # 8 groups per 128 partitions
EPS = 1e-6
SCALE = float(C) ** -0.5
WS = 16.0            # host-side weight scale (power of two)
ESCALE = SCALE / (WS * WS)   # exp() input scale
OSC = 1.0 / 256.0    # O_u -> fp8 shrink, with the 1/WS^2 proj
                     # compensation folded in: out = pj * (1/s) exactly
HC = L // 2          # half-columns per x chunk

AF = mybir.ActivationFunctionType
ALU = mybir.AluOpType
DR = mybir.MatmulPerfMode.DoubleRow


def build_program():
    nc = bacc.Bacc("TRN2")
    x_d = nc.declare_dram_parameter("x", [C, L], F32, isOutput=False)
    wq_d = nc.declare_dram_parameter("wq8", [C, C], F8, isOutput=False)
    wk_d = nc.declare_dram_parameter("wk8", [C, C], F8, isOutput=False)
    wv_d = nc.declare_dram_parameter("wv8", [C, C], F8, isOutput=False)
    wp_d = nc.declare_dram_parameter("wp8", [C, C], F8, isOutput=False)
    gns_d = nc.declare_dram_parameter("gn_scale", [C], F32, isOutput=False)
    gnb_d = nc.declare_dram_parameter("gn_bias", [C], F32, isOutput=False)
    bqw_d = nc.declare_dram_parameter("bqw", [C], F32, isOutput=False)
    bv8_d = nc.declare_dram_parameter("bv8", [C], F8, isOutput=False)
    bp_d = nc.declare_dram_parameter("bp", [C], F32, isOutput=False)
    gm_d = nc.declare_dram_parameter("gmat", [P, GPP], F32, isOutput=False)
    gt_d = nc.declare_dram_parameter("gtmat", [GPP, P], F32, isOutput=False)
    out_d = nc.declare_dram_parameter("out", [C, NQ], F32, isOutput=True)

    with tile.TileContext(nc) as tc:
        attn_block(tc, x_d, wq_d, wk_d, wv_d, wp_d, gns_d, gnb_d,
                   bqw_d, bv8_d, bp_d, gm_d, gt_d, out_d)
    nc.compile()
    return nc


def attn_block(tc, x_d, wq_d, wk_d, wv_d, wp_d, gns_d, gnb_d, bqw_d, bv8_d,
               bp_d, gm_d, gt_d, out_d):
    nc = tc.nc
    x_v = x_d.ap().rearrange("(o p) l -> p o l", p=P)
    out_v = out_d.ap().rearrange("(o p) i -> p o i", p=P)

    with ExitStack() as ctx:
        # ---- persistent pools (whole kernel) ----
        big = ctx.enter_context(tc.tile_pool(name="big", bufs=1))
        small = ctx.enter_context(tc.tile_pool(name="small", bufs=1))

        xq = big.tile([P, CO, NQ], F32, tag="xq")      # resident query half
        x8 = big.tile([P, 2, 2, L], F8, tag="x8")      # GN-normalized x, fp8
        k8 = big.tile([P, 2, 2, L], F8, tag="k8")
        q8 = big.tile([P, 2, 2, NQ], F8, tag="q8")
        vt8 = big.tile([P, NJT // 2, 2, C], F8, tag="vt8")
        wq8 = big.tile([P, 2, 2, C], F8, tag="wq8")
        wk8 = big.tile([P, 2, 2, C], F8, tag="wk8")
        wv8 = big.tile([P, 2, 2, C], F8, tag="wv8")
        wp8 = big.tile([P, 2, 2, C], F8, tag="wp8")

        gns = small.tile([P, CO], F32, tag="gns")
        gnb = small.tile([P, CO], F32, tag="gnb")
        bqw = small.tile([P, CO], F32, tag="bqw")
        bp_s = small.tile([P, CO], F32, tag="bps")
        bp3 = small.tile([P, CO], F32, tag="bp3")
        bv8 = small.tile([P, 2, 2], F8, tag="bv8")
        m44 = small.tile([P, CO], F32, tag="m44")
        a44 = small.tile([P, CO], F32, tag="a44")
        gm_t = small.tile([P, GPP], F32, tag="gmt")
        gt_t = small.tile([GPP, P], F32, tag="gtt")
        ones_p = small.tile([P, 2, P], F8, tag="onesp")
        nc.vector.memset(ones_p, 1.0)
        nshift = small.tile([P, 1], F32, tag="nshift")
        nc.vector.memset(nshift, -3.0)
        eps_t = small.tile([GPP, 1], F32, tag="eps")
        nc.vector.memset(eps_t, EPS)
        warm8 = small.tile([P, 512], F8, tag="warm8")
        nc.vector.memset(warm8, 1.0)

        # static PSUM layout: 3 + 1 + 4 = 8 banks, shared across phases
        ps_st = ctx.enter_context(
            tc.tile_pool(name="psst", bufs=3, space="PSUM"))
        ps_s = ctx.enter_context(
            tc.tile_pool(name="pss", bufs=1, space="PSUM"))
        ps_o = ctx.enter_context(
            tc.tile_pool(name="pso", bufs=4, space="PSUM"))

        # small/weight DMAs on the gpsimd ring (x owns the sync ring)
        for v_d, v_t in ((gns_d, gns), (gnb_d, gnb), (bqw_d, bqw),
                         (bp_d, bp_s)):
            nc.gpsimd.dma_start(out=v_t[:], in_=v_d.ap().rearrange(
                "(o p) -> p o", p=P))
        nc.gpsimd.dma_start(out=bv8[:], in_=bv8_d.ap().rearrange(
            "(a b p) -> p a b", a=2, b=2, p=P))
        nc.gpsimd.dma_start(out=gm_t[:], in_=gm_d.ap())
        nc.gpsimd.dma_start(out=gt_t[:], in_=gt_d.ap())
        for w_d, w_t in ((wk_d, wk8), (wv_d, wv8), (wq_d, wq8), (wp_d, wp8)):
            nc.gpsimd.dma_start(out=w_t[:], in_=w_d.ap().rearrange(
                "(a b p) c -> p a b c", a=2, b=2, p=P))

        # ========== prologue: streamed stats -> GN-fused fp8 cast ==========
        with ExitStack() as pctx:
            xh_pool = pctx.enter_context(tc.tile_pool(name="xhp", bufs=2))
            pro = pctx.enter_context(tc.tile_pool(name="pro", bufs=1))
            tiny_ps = ps_st
            qkv_ps = ps_o

            bnst = pro.tile([P, CO, 8, 6], F32, tag="bnst")
            mv = pro.tile([P, CO, 2], F32, tag="mv")
            st2 = pro.tile([P, CO, 2], F32, tag="st2")
            sc2 = pro.tile([P, CO, 1], F32, tag="sc2")
            grp = pro.tile([GPP, CO, 6], F32, tag="grp")

            xh_tiles = {}
            ci = 0
            for o in range(CO):
                for hh in range(2):
                    l0 = hh * HC
                    if hh == 0:
                        xf = xq[:, o, :]
                    else:
                        xh = xh_pool.tile([P, HC], F32, tag="xh",
                                          name=f"xh{o}")
                        xh_tiles[o] = xh
                        xf = xh[:]
                    # last block arrives in quarters so its stats drain fast
                    npc = 2 if o == CO - 1 else 1
                    for pc in range(npc):
                        w = HC // npc
                        nc.sync.dma_start(
                            out=xf[:, pc * w:(pc + 1) * w],
                            in_=x_v[:, o, l0 + pc * w:l0 + (pc + 1) * w])
                    for h in range(4):
                        nc.vector.bn_stats(
                            out=bnst[:, o, hh * 4 + h, :],
                            in_=xf[:, h * 512:(h + 1) * 512])
                    # HAM warmup: dummy matmul paced by this chunk's stats
                    nc.vector.tensor_copy(warm8[:, ci * 4:ci * 4 + 4],
                                          bnst[:, o, hh * 4 + 3, 0:4])
                    wm_ps = tiny_ps.tile([P, 512], F32, tag="mm")
                    nc.tensor.matmul(wm_ps, lhsT=warm8[:, 0:P],
                                     rhs=warm8[:], start=True, stop=True)
                    ci += 1

                # ---- block o complete: group stats -> m/a -> fp8 cast ----
                nc.vector.bn_aggr(out=mv[:, o, :], in_=bnst[:, o, :, :])
                nc.vector.tensor_copy(st2[:, o, 0:1], mv[:, o, 0:1])
                nc.vector.tensor_mul(sc2[:, o, :], mv[:, o, 0:1],
                                     mv[:, o, 0:1])
                nc.vector.tensor_add(st2[:, o, 1:2], sc2[:, o, :],
                                     mv[:, o, 1:2])
                g_ps = tiny_ps.tile([GPP, 2], F32, tag="mm")
                nc.tensor.matmul(g_ps, lhsT=gm_t, rhs=st2[:, o, :],
                                 start=True, stop=True)
                # var = E[x^2] - mean^2 ; rstd = 1/sqrt(var+eps)
                nc.vector.tensor_copy(grp[:, o, 0:1], g_ps[:, 0:1])
                nc.vector.tensor_mul(grp[:, o, 2:3], grp[:, o, 0:1],
                                     grp[:, o, 0:1])
                nc.vector.tensor_sub(grp[:, o, 2:3], g_ps[:, 1:2],
                                     grp[:, o, 2:3])
                nc.scalar.activation(out=grp[:, o, 3:4], in_=grp[:, o, 2:3],
                                     func=AF.Sqrt, bias=eps_t)
                nc.vector.reciprocal_approx_accurate(
                    grp[:, o, 1:2], grp[:, o, 3:4], grp[:, o, 4:5])
                bc_ps = tiny_ps.tile([P, 2], F32, tag="mm")
                nc.tensor.matmul(bc_ps, lhsT=gt_t, rhs=grp[:, o, 0:2],
                                 start=True, stop=True)
                mcol = m44[:, o:o + 1]
                acol = a44[:, o:o + 1]
                nc.vector.tensor_mul(mcol, bc_ps[:, 1:2], gns[:, o:o + 1])
                nc.vector.tensor_mul(acol, bc_ps[:, 0:1], mcol)
                nc.vector.tensor_sub(acol, gnb[:, o:o + 1], acol)
                # fused GN cast: xhat = m*x + a, all on ACT (DVE owns stats)
                for hh in range(2):
                    src = xq[:, o, :] if hh == 0 else xh_tiles[o][:]
                    dst = x8[:, o // 2, o % 2, hh * HC:(hh + 1) * HC]
                    for pc in range(2):
                        nc.scalar.activation(
                            out=dst[:, pc * 1024:(pc + 1) * 1024],
                            in_=src[:, pc * 1024:(pc + 1) * 1024],
                            func=AF.Identity, bias=acol, scale=mcol)

            # ---- bp3 = bp + (wp @ bv) : post-normalize bias ----
            for oc in range(CO):
                b_ps = tiny_ps.tile([P, 1], F32, tag="mm")
                for o in range(CO):
                    nc.tensor.matmul(b_ps,
                                     lhsT=wp8[:, o // 2, o % 2,
                                              oc * P:(oc + 1) * P],
                                     rhs=bv8[:, o // 2, o % 2:o % 2 + 1],
                                     start=(o == 0), stop=(o == CO - 1))
                nc.vector.tensor_scalar(out=bp3[:, oc:oc + 1], in0=b_ps,
                                        scalar1=1.0 / (WS * WS),
                                        scalar2=bp_s[:, oc:oc + 1],
                                        op0=ALU.mult, op1=ALU.add)

            # ---- Q / K / V^T from resident GN-fp8 x ----
            ev = 0
            for lc in range(NLC):
                l0 = lc * 512
                for oc in range(CO):
                    kp = qkv_ps.tile([P, 512], F32, tag="acc")
                    for pr in range(2):
                        nc.tensor.matmul(
                            kp, lhsT=wk8[:, pr, :, oc * P:(oc + 1) * P],
                            rhs=x8[:, pr, :, l0:l0 + 512],
                            start=(pr == 0), stop=(pr == 1), perf_mode=DR)
                    dst = k8[:, oc // 2, oc % 2, l0:l0 + 512]
                    if ev % 2 == 0:
                        nc.scalar.activation(out=dst, in_=kp, func=AF.Copy)
                    else:
                        nc.vector.tensor_copy(dst, kp)
                    ev += 1
                for jt in range(4):
                    j0 = l0 + jt * P
                    jtg = lc * 4 + jt
                    vp = qkv_ps.tile([P, C], F32, tag="acc")
                    for pr in range(2):
                        nc.tensor.matmul(
                            vp, lhsT=x8[:, pr, :, j0:j0 + P],
                            rhs=wv8[:, pr, :, :],
                            start=(pr == 0), stop=(pr == 1), perf_mode=DR)
                    dst = vt8[:, jtg // 2, jtg % 2, :]
                    if ev % 2 == 0:
                        nc.scalar.activation(out=dst, in_=vp, func=AF.Copy)
                    else:
                        nc.vector.tensor_copy(dst, vp)
                    ev += 1
                if lc < NIB:
                    for oc in range(CO):
                        qp = qkv_ps.tile([P, 512], F32, tag="acc")
                        for pr in range(2):
                            nc.tensor.matmul(
                                qp, lhsT=wq8[:, pr, :, oc * P:(oc + 1) * P],
                                rhs=x8[:, pr, :, l0:l0 + 512],
                                start=(pr == 0), stop=(pr == 1), perf_mode=DR)
                        nc.vector.tensor_scalar_add(
                            q8[:, oc // 2, oc % 2, l0:l0 + 512], qp,
                            bqw[:, oc:oc + 1])

        # ================= attention + proj per i-block =================
        with ExitStack() as actx:
            p_pool = actx.enter_context(tc.tile_pool(name="ppool", bufs=4))
            osb_pool = actx.enter_context(tc.tile_pool(name="osb", bufs=2))
            out_pool = actx.enter_context(tc.tile_pool(name="outp", bufs=4))
            rinv_pool = actx.enter_context(tc.tile_pool(name="rinvp", bufs=2))

            NT = NJT // 2
            steps = [(ib, t) for ib in range(NIB) for t in range(NT)]

            def emit_scores(ib, t):
                """S^T matmuls + exp for step (ib, t) -> p_f8 tile."""
                i0 = ib * 512
                p_f8 = p_pool.tile([P, 2, 512], F8, tag="pbf")
                for ko in range(2):
                    jt = 2 * t + ko
                    st_ps = ps_st.tile([P, 512], F32, tag="mm")
                    for pr in range(2):
                        nc.tensor.matmul(
                            st_ps,
                            lhsT=k8[:, pr, :, jt * P:(jt + 1) * P],
                            rhs=q8[:, pr, :, i0:i0 + 512],
                            start=(pr == 0), stop=(pr == 1), perf_mode=DR)
                    # exp(S/sqrt(c) - 3): shift keeps P in fp8e4 range,
                    # cancels between the s-normalization and bp3 path.
                    nc.scalar.activation(
                        out=p_f8[:, ko, :], in_=st_ps, func=AF.Exp,
                        bias=nshift, scale=ESCALE)
                return p_f8

            s_ps = None
            o_ps = None
            p_cur = emit_scores(0, 0)
            for idx, (ib, t) in enumerate(steps):
                i0 = ib * 512
                if t == 0:
                    s_ps = ps_s.tile([P, 512], F32, tag="srow")
                    o_ps = [ps_o.tile([P, 512], F32, tag="acc",
                                      name=f"oacc{cc}") for cc in range(CO)]
                # prefetch next step's scores: keeps PE fed while this
                # step's exp() drains on ACT
                p_next = (emit_scores(*steps[idx + 1])
                          if idx + 1 < len(steps) else None)
                nc.tensor.matmul(s_ps, lhsT=ones_p, rhs=p_cur,
                                 start=(t == 0), stop=(t == NT - 1),
                                 perf_mode=DR)
                for cc in range(CO):
                    nc.tensor.matmul(
                        o_ps[cc], lhsT=vt8[:, t, :, cc * P:(cc + 1) * P],
                        rhs=p_cur, start=(t == 0), stop=(t == NT - 1),
                        perf_mode=DR)
                p_cur = p_next
                if t < NT - 1:
                    continue
                # ---- epilogue: 1/s, fp8 O, proj, residual ----
                rinv = rinv_pool.tile([P, 512], F32, tag="rinv")
                nc.vector.reciprocal_approx_fast(rinv, s_ps)
                o_sb = osb_pool.tile([P, 2, 2, 512], F8, tag="osb")
                for cc in range(CO):
                    dst = o_sb[:, cc // 2, cc % 2, :]
                    if cc % 2 == 0:
                        nc.scalar.activation(out=dst, in_=o_ps[cc],
                                             func=AF.Copy, scale=OSC)
                    else:
                        nc.vector.tensor_scalar_mul(dst, o_ps[cc], OSC)
                for oc in range(CO):
                    pj_ps = ps_o.tile([P, 512], F32, tag="acc",
                                      name=f"pj{oc}")
                    for pr in range(2):
                        nc.tensor.matmul(
                            pj_ps,
                            lhsT=wp8[:, pr, :, oc * P:(oc + 1) * P],
                            rhs=o_sb[:, pr, :, :],
                            start=(pr == 0), stop=(pr == 1), perf_mode=DR)
                    out_t = out_pool.tile([P, 512], F32, tag="outt")
                    nc.vector.tensor_mul(out_t, pj_ps, rinv)
                    nc.scalar.activation(out=out_t, in_=out_t,
                                         func=AF.Identity,
                                         bias=bp3[:, oc:oc + 1])
                    nc.vector.tensor_add(out_t, out_t,
                                         xq[:, oc, i0:i0 + 512])
                    nc.sync.dma_start(out=out_v[:, oc, i0:i0 + 512],
                                      in_=out_t)


def kernel(**inputs):
    import ml_dtypes

    F8NP = ml_dtypes.float8_e4m3fn
    x = np.ascontiguousarray(np.asarray(inputs["x"], np.float32))
    args = {}
    for nm, w in (("wq8", inputs["wq"]), ("wk8", inputs["wk"]),
                  ("wv8", inputs["wv"]), ("wp8", inputs["wp"])):
        wT = np.asarray(w, np.float32).T * WS
        args[nm] = np.ascontiguousarray(wT.astype(F8NP))
    args["gn_scale"] = np.asarray(inputs["gn_scale"], np.float32)
    args["gn_bias"] = np.asarray(inputs["gn_bias"], np.float32)
    args["bqw"] = np.asarray(inputs["bq"], np.float32) * np.float32(WS)
    args["bv8"] = (np.asarray(inputs["bv"], np.float32)
                   * np.float32(WS)).astype(F8NP)
    args["bp"] = np.asarray(inputs["bp"], np.float32)
    pidx = np.arange(P)
    gmat = (pidx[:, None] // GSZ == np.arange(GPP)[None, :]).astype(np.float32)
    args["gmat"] = np.ascontiguousarray(gmat / float(GSZ))
    args["gtmat"] = np.ascontiguousarray(gmat.T)
    in_maps = []
    for core in range(8):
        bi, half = core // 2, core % 2
        sl = slice(half * NQ, (half + 1) * NQ)
        other = slice((1 - half) * NQ, (2 - half) * NQ)
        xp = np.ascontiguousarray(
            np.concatenate([x[bi][:, sl], x[bi][:, other]], axis=1))
        in_maps.append({"x": xp, **args})

    from concourse.bass_utils import run_bass_kernel_spmd

    nc = build_program()
    trace = bool(int(os.environ.get("KERNEL_TRACE", "0")))
    res = run_bass_kernel_spmd(nc, in_maps, core_ids=list(range(8)),
                               trace=trace)
    kernel.last_results = res
    out = np.empty((B, C, L), np.float32)
    for core in range(8):
        bi, half = core // 2, core % 2
        out[bi][:, half * NQ:(half + 1) * NQ] = res.results[core]["out"]
    return out


# revision 14
# speedup vs baseline: 1.4556x; 1.0416x over previous
"""Trainium2 Bass kernel for nn_AttnBlock (GroupNorm + single-head attention
block over [b=4, c=512, l=4096] fp32, 8 NeuronCores).

Sharding: core = (batch, query-half). Each core gets one batch item with its
query half permuted to columns 0..2047 (GroupNorm/attention are invariant to
a consistent permutation of l), computes the full block for its 2048 query
positions, and the host reassembles the [4, 512, 4096] output.

v2 design (vs baseline):
  - Weights pre-scaled by WS=16 and cast to fp8e4m3 on the HOST: kills 3 MB
    of prologue DMA + all on-chip weight-fold matmuls/casts. Power-of-two
    compensations fold into existing scalar constants.
  - GroupNorm applied to x during the fp8 cast (scale+bias fused into
    ACT Identity / DVE tensor_scalar), per channel-block pipelined stats:
    no weight folding, no bias-fixup matmuls, QKV starts ~20 us earlier.
  - x query-half kept resident in SBUF (f32) for the residual: no re-read.
  - s row-sums via M=128 all-ones stationary => s broadcast to all
    partitions for free; 1/s on DVE directly; no PE broadcast matmul, no
    rank-1 bias-inject matmuls (bp3 added post-normalize on ACT).
  - PE warmup dummies paced by stats chunks to pre-trigger the HAM clock
    un-throttle before the QKV burst.
"""
import os
import sys
from contextlib import ExitStack

import numpy as np

sys.path.insert(0, "/opt/trn_rl_repo")

import concourse.bass as bass
import concourse.tile as tile
from concourse import bacc, mybir

F32 = mybir.dt.float32
BF16 = mybir.dt.bfloat16
F8 = mybir.dt.float8e4

B, C, L = 4, 512, 4096
NQ = L // 2          # queries per core
P = 128
CO = C // P          # 4 channel blocks
NJT = L // P         # 32 j-tiles
NIB = NQ // 512      # 4 i-blocks
NLC = L // 512       # 8 l-chunks
NG = 32              # groups
GSZ = C // NG        # 16 channels per group
GPP = P // GSZ       # 8 groups per 128 partitions
EPS = 1e-6
SCALE = float(C) ** -0.5
WS = 16.0            # host-side weight scale (power of two)
ESCALE = SCALE / (WS * WS)   # exp() input scale
OSC = 1.0 / 256.0    # O_u -> fp8 shrink, with the 1/WS^2 proj
                     # compensation folded in: out = pj * (1/s) exactly
HC = L // 2          # half-columns per x chunk

AF = mybir.ActivationFunctionType
ALU = mybir.AluOpType
DR = mybir.MatmulPerfMode.DoubleRow


def build_program():
    nc = bacc.Bacc("TRN2")
    x_d = nc.declare_dram_parameter("xb", [C, L], BF16, isOutput=False)
    xr_d = nc.declare_dram_parameter("xr", [C, NQ], F32, isOutput=False)
    wq_d = nc.declare_dram_parameter("wq8", [C, C], F8, isOutput=False)
    wk_d = nc.declare_dram_parameter("wk8", [C, C], F8, isOutput=False)
    wv_d = nc.declare_dram_parameter("wv8", [C, C], F8, isOutput=False)
    wp_d = nc.declare_dram_parameter("wp8", [C, C], F8, isOutput=False)
    gns_d = nc.declare_dram_parameter("gn_scale", [C], F32, isOutput=False)
    gnb_d = nc.declare_dram_parameter("gn_bias", [C], F32, isOutput=False)
    bqw_d = nc.declare_dram_parameter("bqw", [C], F32, isOutput=False)
    bv8_d = nc.declare_dram_parameter("bv8", [C], F8, isOutput=False)
    bp_d = nc.declare_dram_parameter("bp", [C], F32, isOutput=False)
    gm_d = nc.declare_dram_parameter("gmat", [P, GPP], F32, isOutput=False)
    gt_d = nc.declare_dram_parameter("gtmat", [GPP, P], F32, isOutput=False)
    out_d = nc.declare_dram_parameter("out", [C, NQ], F32, isOutput=True)

    with tile.TileContext(nc) as tc:
        attn_block(tc, x_d, xr_d, wq_d, wk_d, wv_d, wp_d, gns_d, gnb_d,
                   bqw_d, bv8_d, bp_d, gm_d, gt_d, out_d)
    nc.compile()
    return nc


def attn_block(tc, x_d, xr_d, wq_d, wk_d, wv_d, wp_d, gns_d, gnb_d,
               bqw_d, bv8_d, bp_d, gm_d, gt_d, out_d):
    nc = tc.nc
    x_v = x_d.ap().rearrange("(o p) l -> p o l", p=P)
    xr_v = xr_d.ap().rearrange("(o p) i -> p o i", p=P)
    out_v = out_d.ap().rearrange("(o p) i -> p o i", p=P)

    with ExitStack() as ctx:
        # ---- persistent pools (whole kernel) ----
        big = ctx.enter_context(tc.tile_pool(name="big", bufs=1))
        small = ctx.enter_context(tc.tile_pool(name="small", bufs=1))

        xb_t = big.tile([P, CO, L], BF16, tag="xbt")   # bf16 x for stats/cast
        x8 = big.tile([P, 2, 2, L], F8, tag="x8")      # GN-normalized x, fp8
        k8 = big.tile([P, 2, 2, L], F8, tag="k8")
        q8 = big.tile([P, 2, 2, NQ], F8, tag="q8")
        vt8 = big.tile([P, NJT // 2, 2, C], F8, tag="vt8")
        wq8 = big.tile([P, 2, 2, C], F8, tag="wq8")
        wk8 = big.tile([P, 2, 2, C], F8, tag="wk8")
        wv8 = big.tile([P, 2, 2, C], F8, tag="wv8")
        wp8 = big.tile([P, 2, 2, C], F8, tag="wp8")

        gns = small.tile([P, CO], F32, tag="gns")
        gnb = small.tile([P, CO], F32, tag="gnb")
        bqw = small.tile([P, CO], F32, tag="bqw")
        bp_s = small.tile([P, CO], F32, tag="bps")
        bp3 = small.tile([P, CO], F32, tag="bp3")
        bv8 = small.tile([P, 2, 2], F8, tag="bv8")
        m44 = small.tile([P, CO], F32, tag="m44")
        a44 = small.tile([P, CO], F32, tag="a44")
        gm_t = small.tile([P, GPP], F32, tag="gmt")
        gt_t = small.tile([GPP, P], F32, tag="gtt")
        ones_p = small.tile([P, 2, P], F8, tag="onesp")
        nc.vector.memset(ones_p, 1.0)
        nshift = small.tile([P, 1], F32, tag="nshift")
        nc.vector.memset(nshift, -3.0)
        eps_t = small.tile([GPP, 1], F32, tag="eps")
        nc.vector.memset(eps_t, EPS)
        warm8 = small.tile([P, 512], F8, tag="warm8")
        nc.vector.memset(warm8, 1.0)

        # static PSUM layout: 3 + 1 + 4 = 8 banks, shared across phases
        ps_st = ctx.enter_context(
            tc.tile_pool(name="psst", bufs=3, space="PSUM"))
        ps_s = ctx.enter_context(
            tc.tile_pool(name="pss", bufs=1, space="PSUM"))
        ps_o = ctx.enter_context(
            tc.tile_pool(name="pso", bufs=4, space="PSUM"))

        # small/weight DMAs on the gpsimd ring (x owns the sync ring)
        for v_d, v_t in ((gns_d, gns), (gnb_d, gnb), (bqw_d, bqw),
                         (bp_d, bp_s)):
            nc.gpsimd.dma_start(out=v_t[:], in_=v_d.ap().rearrange(
                "(o p) -> p o", p=P))
        nc.gpsimd.dma_start(out=bv8[:], in_=bv8_d.ap().rearrange(
            "(a b p) -> p a b", a=2, b=2, p=P))
        nc.gpsimd.dma_start(out=gm_t[:], in_=gm_d.ap())
        nc.gpsimd.dma_start(out=gt_t[:], in_=gt_d.ap())
        for w_d, w_t in ((wk_d, wk8), (wv_d, wv8), (wq_d, wq8), (wp_d, wp8)):
            nc.gpsimd.dma_start(out=w_t[:], in_=w_d.ap().rearrange(
                "(a b p) c -> p a b c", a=2, b=2, p=P))

        # ========== prologue: streamed stats -> GN-fused fp8 cast ==========
        with ExitStack() as pctx:
            pro = pctx.enter_context(tc.tile_pool(name="pro", bufs=1))
            tiny_ps = ps_st
            qkv_ps = ps_o

            bnst = pro.tile([P, CO, 8, 6], F32, tag="bnst")
            mv = pro.tile([P, CO, 2], F32, tag="mv")
            st2 = pro.tile([P, CO, 2], F32, tag="st2")
            sc2 = pro.tile([P, CO, 1], F32, tag="sc2")
            grp = pro.tile([GPP, CO, 6], F32, tag="grp")

            ci = 0
            for o in range(CO):
                for hh in range(2):
                    l0 = hh * HC
                    xf = xb_t[:, o, l0:l0 + HC]
                    # last block arrives in quarters so its stats drain fast
                    npc = 2 if o == CO - 1 else 1
                    for pc in range(npc):
                        w = HC // npc
                        nc.sync.dma_start(
                            out=xf[:, pc * w:(pc + 1) * w],
                            in_=x_v[:, o, l0 + pc * w:l0 + (pc + 1) * w])
                    for h in range(4):
                        nc.vector.bn_stats(
                            out=bnst[:, o, hh * 4 + h, :],
                            in_=xf[:, h * 512:(h + 1) * 512])
                    # HAM warmup: dummy matmul paced by this chunk's stats
                    nc.vector.tensor_copy(warm8[:, ci * 4:ci * 4 + 4],
                                          bnst[:, o, hh * 4 + 3, 0:4])
                    wm_ps = tiny_ps.tile([P, 512], F32, tag="mm")
                    nc.tensor.matmul(wm_ps, lhsT=warm8[:, 0:P],
                                     rhs=warm8[:], start=True, stop=True)
                    ci += 1

                # ---- block o complete: group stats -> m/a -> fp8 cast ----
                nc.vector.bn_aggr(out=mv[:, o, :], in_=bnst[:, o, :, :])
                nc.vector.tensor_copy(st2[:, o, 0:1], mv[:, o, 0:1])
                nc.vector.tensor_mul(sc2[:, o, :], mv[:, o, 0:1],
                                     mv[:, o, 0:1])
                nc.vector.tensor_add(st2[:, o, 1:2], sc2[:, o, :],
                                     mv[:, o, 1:2])
                g_ps = tiny_ps.tile([GPP, 2], F32, tag="mm")
                nc.tensor.matmul(g_ps, lhsT=gm_t, rhs=st2[:, o, :],
                                 start=True, stop=True)
                # var = E[x^2] - mean^2 ; rstd = 1/sqrt(var+eps)
                nc.vector.tensor_copy(grp[:, o, 0:1], g_ps[:, 0:1])
                nc.vector.tensor_mul(grp[:, o, 2:3], grp[:, o, 0:1],
                                     grp[:, o, 0:1])
                nc.vector.tensor_sub(grp[:, o, 2:3], g_ps[:, 1:2],
                                     grp[:, o, 2:3])
                nc.scalar.activation(out=grp[:, o, 3:4], in_=grp[:, o, 2:3],
                                     func=AF.Sqrt, bias=eps_t)
                nc.vector.reciprocal_approx_accurate(
                    grp[:, o, 1:2], grp[:, o, 3:4], grp[:, o, 4:5])
                bc_ps = tiny_ps.tile([P, 2], F32, tag="mm")
                nc.tensor.matmul(bc_ps, lhsT=gt_t, rhs=grp[:, o, 0:2],
                                 start=True, stop=True)
                mcol = m44[:, o:o + 1]
                acol = a44[:, o:o + 1]
                nc.vector.tensor_mul(mcol, bc_ps[:, 1:2], gns[:, o:o + 1])
                nc.vector.tensor_mul(acol, bc_ps[:, 0:1], mcol)
                nc.vector.tensor_sub(acol, gnb[:, o:o + 1], acol)
                # fused GN cast: xhat = m*x + a (3/4 on ACT, 1/4 on DVE)
                for pc in range(4):
                    srcp = xb_t[:, o, pc * 1024:(pc + 1) * 1024]
                    dst = x8[:, o // 2, o % 2, pc * 1024:(pc + 1) * 1024]
                    if pc == 3:
                        nc.vector.tensor_scalar(
                            out=dst, in0=srcp, scalar1=mcol, scalar2=acol,
                            op0=ALU.mult, op1=ALU.add)
                    else:
                        nc.scalar.activation(out=dst, in_=srcp,
                                             func=AF.Identity,
                                             bias=acol, scale=mcol)

            # ---- bp3 = bp + (wp @ bv) : post-normalize bias ----
            for oc in range(CO):
                b_ps = tiny_ps.tile([P, 1], F32, tag="mm")
                for o in range(CO):
                    nc.tensor.matmul(b_ps,
                                     lhsT=wp8[:, o // 2, o % 2,
                                              oc * P:(oc + 1) * P],
                                     rhs=bv8[:, o // 2, o % 2:o % 2 + 1],
                                     start=(o == 0), stop=(o == CO - 1))
                nc.vector.tensor_scalar(out=bp3[:, oc:oc + 1], in0=b_ps,
                                        scalar1=1.0 / (WS * WS),
                                        scalar2=bp_s[:, oc:oc + 1],
                                        op0=ALU.mult, op1=ALU.add)

            # ---- Q / K / V^T from resident GN-fp8 x ----
            ev = 0
            for lc in range(NLC):
                l0 = lc * 512
                for oc in range(CO):
                    kp = qkv_ps.tile([P, 512], F32, tag="acc")
                    for pr in range(2):
                        nc.tensor.matmul(
                            kp, lhsT=wk8[:, pr, :, oc * P:(oc + 1) * P],
                            rhs=x8[:, pr, :, l0:l0 + 512],
                            start=(pr == 0), stop=(pr == 1), perf_mode=DR)
                    dst = k8[:, oc // 2, oc % 2, l0:l0 + 512]
                    if ev % 2 == 0:
                        nc.scalar.activation(out=dst, in_=kp, func=AF.Copy)
                    else:
                        nc.vector.tensor_copy(dst, kp)
                    ev += 1
                for jt in range(4):
                    j0 = l0 + jt * P
                    jtg = lc * 4 + jt
                    vp = qkv_ps.tile([P, C], F32, tag="acc")
                    for pr in range(2):
                        nc.tensor.matmul(
                            vp, lhsT=x8[:, pr, :, j0:j0 + P],
                            rhs=wv8[:, pr, :, :],
                            start=(pr == 0), stop=(pr == 1), perf_mode=DR)
                    dst = vt8[:, jtg // 2, jtg % 2, :]
                    if ev % 2 == 0:
                        nc.scalar.activation(out=dst, in_=vp, func=AF.Copy)
                    else:
                        nc.vector.tensor_copy(dst, vp)
                    ev += 1
                if lc < NIB:
                    for oc in range(CO):
                        qp = qkv_ps.tile([P, 512], F32, tag="acc")
                        for pr in range(2):
                            nc.tensor.matmul(
                                qp, lhsT=wq8[:, pr, :, oc * P:(oc + 1) * P],
                                rhs=x8[:, pr, :, l0:l0 + 512],
                                start=(pr == 0), stop=(pr == 1), perf_mode=DR)
                        nc.vector.tensor_scalar_add(
                            q8[:, oc // 2, oc % 2, l0:l0 + 512], qp,
                            bqw[:, oc:oc + 1])

        # ================= attention + proj per i-block =================
        with ExitStack() as actx:
            p_pool = actx.enter_context(tc.tile_pool(name="ppool", bufs=4))
            res_pool = actx.enter_context(tc.tile_pool(name="resp", bufs=2))
            osb_pool = actx.enter_context(tc.tile_pool(name="osb", bufs=2))
            out_pool = actx.enter_context(tc.tile_pool(name="outp", bufs=4))
            rinv_pool = actx.enter_context(tc.tile_pool(name="rinvp", bufs=2))

            NT = NJT // 2
            steps = [(ib, t) for ib in range(NIB) for t in range(NT)]

            def emit_scores(ib, t):
                """S^T matmuls + exp for step (ib, t) -> p_f8 tile."""
                i0 = ib * 512
                p_f8 = p_pool.tile([P, 2, 512], F8, tag="pbf")
                for ko in range(2):
                    jt = 2 * t + ko
                    st_ps = ps_st.tile([P, 512], F32, tag="mm")
                    for pr in range(2):
                        nc.tensor.matmul(
                            st_ps,
                            lhsT=k8[:, pr, :, jt * P:(jt + 1) * P],
                            rhs=q8[:, pr, :, i0:i0 + 512],
                            start=(pr == 0), stop=(pr == 1), perf_mode=DR)
                    # exp(S/sqrt(c) - 3): shift keeps P in fp8e4 range,
                    # cancels between the s-normalization and bp3 path.
                    nc.scalar.activation(
                        out=p_f8[:, ko, :], in_=st_ps, func=AF.Exp,
                        bias=nshift, scale=ESCALE)
                return p_f8

            s_ps = None
            o_ps = None
            p_cur = emit_scores(0, 0)
            for idx, (ib, t) in enumerate(steps):
                i0 = ib * 512
                if t == 0:
                    s_ps = ps_s.tile([P, 512], F32, tag="srow")
                    o_ps = [ps_o.tile([P, 512], F32, tag="acc",
                                      name=f"oacc{cc}") for cc in range(CO)]
                    res = res_pool.tile([P, CO, 512], F32, tag="res")
                    nc.sync.dma_start(out=res[:],
                                      in_=xr_v[:, :, i0:i0 + 512])
                # prefetch next step's scores: keeps PE fed while this
                # step's exp() drains on ACT
                p_next = (emit_scores(*steps[idx + 1])
                          if idx + 1 < len(steps) else None)
                nc.tensor.matmul(s_ps, lhsT=ones_p, rhs=p_cur,
                                 start=(t == 0), stop=(t == NT - 1),
                                 perf_mode=DR)
                for cc in range(CO):
                    nc.tensor.matmul(
                        o_ps[cc], lhsT=vt8[:, t, :, cc * P:(cc + 1) * P],
                        rhs=p_cur, start=(t == 0), stop=(t == NT - 1),
                        perf_mode=DR)
                p_cur = p_next
                if t < NT - 1:
                    continue
                # ---- epilogue: 1/s, fp8 O, proj, residual ----
                rinv = rinv_pool.tile([P, 512], F32, tag="rinv")
                nc.vector.reciprocal_approx_fast(rinv, s_ps)
                o_sb = osb_pool.tile([P, 2, 2, 512], F8, tag="osb")
                for cc in range(CO):
                    dst = o_sb[:, cc // 2, cc % 2, :]
                    if cc % 2 == 0:
                        nc.scalar.activation(out=dst, in_=o_ps[cc],
                                             func=AF.Copy, scale=OSC)
                    else:
                        nc.vector.tensor_scalar_mul(dst, o_ps[cc], OSC)
                for oc in range(CO):
                    pj_ps = ps_o.tile([P, 512], F32, tag="acc",
                                      name=f"pj{oc}")
                    for pr in range(2):
                        nc.tensor.matmul(
                            pj_ps,
                            lhsT=wp8[:, pr, :, oc * P:(oc + 1) * P],
                            rhs=o_sb[:, pr, :, :],
                            start=(pr == 0), stop=(pr == 1), perf_mode=DR)
                    out_t = out_pool.tile([P, 512], F32, tag="outt")
                    nc.vector.tensor_mul(out_t, pj_ps, rinv)
                    nc.scalar.activation(out=out_t, in_=out_t,
                                         func=AF.Identity,
                                         bias=bp3[:, oc:oc + 1])
                    nc.vector.tensor_add(out_t, out_t, res[:, oc, :])
                    nc.sync.dma_start(out=out_v[:, oc, i0:i0 + 512],
                                      in_=out_t)


def kernel(**inputs):
    import ml_dtypes

    F8NP = ml_dtypes.float8_e4m3fn
    BF16NP = ml_dtypes.bfloat16
    x = np.ascontiguousarray(np.asarray(inputs["x"], np.float32))
    args = {}
    for nm, w in (("wq8", inputs["wq"]), ("wk8", inputs["wk"]),
                  ("wv8", inputs["wv"]), ("wp8", inputs["wp"])):
        wT = np.asarray(w, np.float32).T * WS
        args[nm] = np.ascontiguousarray(wT.astype(F8NP))
    args["gn_scale"] = np.asarray(inputs["gn_scale"], np.float32)
    args["gn_bias"] = np.asarray(inputs["gn_bias"], np.float32)
    args["bqw"] = np.asarray(inputs["bq"], np.float32) * np.float32(WS)
    args["bv8"] = (np.asarray(inputs["bv"], np.float32)
                   * np.float32(WS)).astype(F8NP)
    args["bp"] = np.asarray(inputs["bp"], np.float32)
    pidx = np.arange(P)
    gmat = (pidx[:, None] // GSZ == np.arange(GPP)[None, :]).astype(np.float32)
    args["gmat"] = np.ascontiguousarray(gmat / float(GSZ))
    args["gtmat"] = np.ascontiguousarray(gmat.T)
    in_maps = []
    for core in range(8):
        bi, half = core // 2, core % 2
        sl = slice(half * NQ, (half + 1) * NQ)
        other = slice((1 - half) * NQ, (2 - half) * NQ)
        xp = np.concatenate([x[bi][:, sl], x[bi][:, other]], axis=1)
        in_maps.append({"xb": np.ascontiguousarray(xp.astype(BF16NP)),
                        "xr": np.ascontiguousarray(x[bi][:, sl]), **args})

    from concourse.bass_utils import run_bass_kernel_spmd

    nc = build_program()
    trace = bool(int(os.environ.get("KERNEL_TRACE", "0")))
    res = run_bass_kernel_spmd(nc, in_maps, core_ids=list(range(8)),
                               trace=trace)
    kernel.last_results = res
    out = np.empty((B, C, L), np.float32)
    for core in range(8):
        bi, half = core // 2, core % 2
        out[bi][:, half * NQ:(half + 1) * NQ] = res.results[core]["out"]
    return out
